# revision 4
# baseline (speedup 1.0000x reference)
"""GATv2ConvNet on 8 Trainium2 NeuronCores (Bass SPMD kernel).

Strategy: partition nodes (and incident edges, sorted by destination) across
the 8 cores; replicate the small GATv2/MLP weights; AllGather fp16 xl feature
tables between layers; AllReduce the per-graph pooled sums.

v2 inner loop: per edge tile, dma_gather fetches BOTH xl[src] (from the
allgathered table) and xr[dst] (from the core-local xr table) rows from HBM;
the selection matrices S^T are precomputed on host and DMA-loaded; DVE does
only 4 F-wide ops per edge (add, att-mult, logit-reduce, p-mult); ScalarE does
LeakyReLU+Exp; the scatter-add into per-node-block PSUM stays a matmul with
the S^T slabs as stationary operands.
"""

import os
import sys
import functools
import numpy as np

for _p in ("/opt/trn_rl_repo",):
    if _p not in sys.path:
        sys.path.insert(0, _p)

NC = 8
N = 30000
V = 10000
G = 128
P = 128
NPC = N // NC            # 3750 nodes per core
NBLK = 30                # node blocks of 128 per core
CHUNKS = [14, 10, 6]     # node blocks per table-allgather chunk (front-loaded)
CHB = [0, 14, 24, 30]    # chunk block boundaries
TCH = 10                 # (unused; kept for compat)
NCH = len(CHUNKS)
NSLOT = NBLK * P         # 3840
NEG = 0.2
SB_BUFS = 2
GCK = 8                  # tiles per gather instruction (1024 idxs; ni>=2048 crashes HW)
GATH_BUFS = 3
ST_BUFS = 3
OUT_BUFS = 4

LAYERS = [
    dict(inF=16, F=96, H=3, C=32, elem=128),
    dict(inF=96, F=192, H=2, C=96, elem=256),
    dict(inF=192, F=64, H=1, C=64, elem=128),
]

f16 = np.float16
f32 = np.float32


def _wrap16(idx):
    """dma_gather index layout: [128, n/16] int16; index i at [i%16, i//16]."""
    n = idx.shape[0]
    assert n % 16 == 0
    w = idx.reshape(n // 16, 16).T.astype(np.int16)
    return np.tile(w, (8, 1))


def _host_prep(inputs):
    node_ids = np.asarray(inputs["node_ids"], np.int64)
    edge_index = np.asarray(inputs["edge_index"], np.int64)
    batch = np.asarray(inputs["batch"], np.int64)

    loops = np.arange(N, dtype=np.int64)
    src = np.concatenate([edge_index[0], loops])
    dst = np.concatenate([edge_index[1], loops])
    order = np.argsort(dst, kind="stable")
    src = src[order]
    dst = dst[order]

    per_core = []
    t_blk = 0
    for c in range(NC):
        sel = (dst >= c * NPC) & (dst < (c + 1) * NPC)
        s_c, d_c = src[sel], dst[sel] - c * NPC
        blocks = []
        for b in range(NBLK):
            m = (d_c // P) == b
            blocks.append((s_c[m], d_c[m] % P))
            t_blk = max(t_blk, (len(blocks[-1][0]) + P - 1) // P)
        per_core.append(blocks)

    NT = NBLK * t_blk
    EP = NT * P

    counts_g = np.bincount(batch, minlength=G).astype(f32)

    cores = []
    for c in range(NC):
        esrc = np.zeros(EP, np.int64)
        dloc = np.full(EP, -1, np.int64)
        for b, (s_b, d_b) in enumerate(per_core[c]):
            o = b * t_blk * P
            n_b = len(s_b)
            esrc[o:o + n_b] = s_b
            dloc[o:o + n_b] = d_b
        idx0 = _wrap16(node_ids[esrc])
        c_s = esrc // NPC
        l_s = esrc % NPC
        idx1 = _wrap16(c_s * NSLOT + l_s)            # xl1pad: [core][slot] layout
        blk_s = l_s // P                             # node block of the src node
        k_s = np.searchsorted(np.asarray(CHB), blk_s, side="right") - 1
        chb = np.asarray(CHB)[k_s]                   # first block of that chunk
        chw = np.asarray(CHUNKS)[k_s]                # blocks in that chunk
        choff = np.asarray([0] + list(np.cumsum(np.asarray(CHUNKS) * NC * P)[:-1]))
        idx2 = _wrap16(choff[k_s] + c_s * (chw * P) + (l_s - chb * P))
        dclamp = np.maximum(dloc, 0)
        # xr-gather indices: layer 0 reads embr rows by vocab id of the dst
        # node; layers 1-2 read the core-local xr table by local dst slot.
        blk_of = np.repeat(np.arange(NBLK), t_blk * P)
        dslot = blk_of * P + dclamp
        idxr0 = _wrap16(node_ids[c * NPC + np.minimum(dslot, NPC - 1)])
        idxr12 = _wrap16(dslot)
        # S^T slabs: stT[p_edge, t, n] = 1 iff dst of edge slot (t, p) == n
        dcol = dloc.reshape(NT, P).T                 # [128, NT]
        stT = (dcol[:, :, None] == np.arange(P)[None, None, :]).astype(f16)
        stT = np.ascontiguousarray(stT.reshape(P, NT * P))

        bl = np.full(NSLOT, -1.0, f32)
        bl[:NPC] = batch[c * NPC:(c + 1) * NPC].astype(f32)
        batchcol = np.ascontiguousarray(bl.reshape(NBLK, P).T).astype(f16)

        dstcol = np.ascontiguousarray(dloc.reshape(NT, P).T).astype(f16)
        sgm = (bl.reshape(NBLK, P)[:, :, None] ==
               np.arange(G, dtype=f32)[None, None, :])
        sgm = sgm * (1.0 / np.maximum(counts_g, 1.0))[None, None, :]
        sgmat = np.ascontiguousarray(sgm.transpose(1, 0, 2).reshape(P, NBLK * G)).astype(f16)
        cores.append(dict(idx0=idx0, idx1=idx1, idx2=idx2, idxr0=idxr0,
                          idxr12=idxr12, stT=stT, batchcol=batchcol,
                          dstcol=dstcol, sgmat=sgmat))


    shared = {}
    embp = np.zeros((10240, 16), f32)
    embp[:V] = np.asarray(inputs["emb"], f32)
    shared["embt"] = np.ascontiguousarray(embp.T.astype(f16))
    for l in range(3):
        F = LAYERS[l]["F"]
        shared[f"wl{l}"] = np.asarray(inputs[f"Wl{l}"], f32).astype(f16)
        shared[f"wr{l}"] = np.asarray(inputs[f"Wr{l}"], f32).astype(f16)
        shared[f"attrep{l}"] = np.tile(
            np.asarray(inputs[f"att{l}"], f32).reshape(1, F), (P, 1)).astype(f16)
        shared[f"blrep{l}"] = np.tile(
            np.asarray(inputs[f"bl{l}"], f32).reshape(1, F), (P, 1)).astype(f16)
        shared[f"brrep{l}"] = np.tile(
            np.asarray(inputs[f"br{l}"], f32).reshape(1, F), (P, 1)).astype(f16)
        shared[f"borep{l}"] = np.tile(
            np.asarray(inputs[f"bo{l}"], f32).reshape(1, F), (P, 1)).astype(f16)
    shared["wl1b"] = np.ascontiguousarray(np.vstack(
        [np.asarray(inputs["Wl1"], f32), np.asarray(inputs["bl1"], f32).reshape(1, -1)]
    ).astype(f16))
    shared["wr1b"] = np.ascontiguousarray(np.vstack(
        [np.asarray(inputs["Wr1"], f32), np.asarray(inputs["br1"], f32).reshape(1, -1)]
    ).astype(f16))
    shared["wl2a"] = shared["wl2"][:P]
    shared["wl2b"] = shared["wl2"][P:]
    shared["wr2a"] = shared["wr2"][:P]
    shared["wr2b"] = shared["wr2"][P:]
    del shared["wl2"], shared["wr2"]
    shared["ident"] = np.eye(P, dtype=f16)
    shared["iota"] = np.tile(np.arange(P, dtype=f16).reshape(1, P), (P, 1))
    shared["wc1"] = np.asarray(inputs["Wc1"], f32).astype(f16)
    shared["wc2"] = np.asarray(inputs["Wc2"], f32).astype(f16)
    shared["bc1col"] = np.ascontiguousarray(np.asarray(inputs["bc1"], f32).reshape(32, 1))
    shared["bc2col"] = np.ascontiguousarray(np.asarray(inputs["bc2"], f32).reshape(2, 1))
    shared["demot"] = np.ascontiguousarray(
        np.asarray(inputs["demographics"], f32).astype(f16).T)

    return t_blk, cores, shared


@functools.lru_cache(maxsize=2)
def _build_program(t_blk):
    import concourse.bass as bass
    import concourse.mybir as mybir
    import concourse.tile as tile
    from concourse import bacc
    from contextlib import ExitStack

    dt = mybir.dt
    Alu = mybir.AluOpType
    Act = mybir.ActivationFunctionType
    NT = NBLK * t_blk
    EP = NT * P
    NPAIR = NBLK // 2
    PAIR_E = 2 * t_blk * P

    nc = bacc.Bacc()

    def par(name, shape, dtype, out=False):
        return nc.declare_dram_parameter(name, list(shape), dtype, isOutput=out)

    embt_p = par("embt", (16, 10240), dt.float16)
    wl_p = [par(f"wl{l}", (LAYERS[l]["inF"], LAYERS[l]["F"]), dt.float16) for l in range(2)]
    wr_p = [par(f"wr{l}", (LAYERS[l]["inF"], LAYERS[l]["F"]), dt.float16) for l in range(2)]
    w2_p = {k: par(k, (P if k.endswith("a") else 64, 64), dt.float16)
            for k in ("wl2a", "wl2b", "wr2a", "wr2b")}
    attr_p = [par(f"attrep{l}", (P, LAYERS[l]["F"]), dt.float16) for l in range(3)]
    blr_p = [par(f"blrep{l}", (P, LAYERS[l]["F"]), dt.float16) for l in range(3)]
    brr_p = [par(f"brrep{l}", (P, LAYERS[l]["F"]), dt.float16) for l in range(3)]
    bor_p = [par(f"borep{l}", (P, LAYERS[l]["F"]), dt.float16) for l in range(3)]
    ident_p = par("ident", (P, P), dt.float16)
    iota_p = par("iota", (P, P), dt.float16)
    dstcol_p = par("dstcol", (P, NT), dt.float16)
    sgmat_p = par("sgmat", (P, NBLK * G), dt.float16)
    idx0_p = par("idx0", (P, EP // 16), dt.int16)
    idx1_p = par("idx1", (P, EP // 16), dt.int16)
    idx2_p = par("idx2", (P, EP // 16), dt.int16)
    idxr0_p = par("idxr0", (P, EP // 16), dt.int16)
    wl1b_p = par("wl1b", (97, 192), dt.float16)
    wr1b_p = par("wr1b", (97, 192), dt.float16)
    idxr12_p = par("idxr12", (P, EP // 16), dt.int16)
    stT_p = par("stT", (P, EP), dt.float16)
    out_p = par("out", (64, G), dt.float32, out=True)

    embl_d = nc.dram_tensor("embl_d", [10240, 128], dt.float16)
    embr_d = nc.dram_tensor("embr_d", [10240, 128], dt.float16)
    x1ownT_d = [nc.dram_tensor(f"x1ownT{k}", [96, CHUNKS[k] * P], dt.float16)
                for k in range(NCH)]
    x1fullT_d = [nc.dram_tensor(f"x1fullT{k}", [NC * 96, CHUNKS[k] * P], dt.float16,
                                addr_space="Shared") for k in range(NCH)]
    xlown_d = [None,
               None,
               nc.dram_tensor("xl2own", [NSLOT, 64], dt.float16)]
    xrown_d = [None,
               nc.dram_tensor("xr1own", [NSLOT, 256], dt.float16),
               nc.dram_tensor("xr2own", [NSLOT, 128], dt.float16)]
    xlfull_d = [None,
                None,
                nc.dram_tensor("xl2full", [NC * NSLOT, 64], dt.float16,
                               addr_space="Shared")]
    xlpad_d = [None,
               nc.dram_tensor("xl1pad", [NC * NSLOT, 256], dt.float16),
               nc.dram_tensor("xl2pad", [NC * NSLOT, 128], dt.float16)]

    replica_groups = [list(range(NC))]

    with ExitStack() as ctx:
        tc = ctx.enter_context(tile.TileContext(nc))
        ctx.enter_context(nc.allow_low_precision("fp16 gat pipeline"))
        cpool = ctx.enter_context(tc.tile_pool(name="consts", bufs=1))
        ppool = ctx.enter_context(tc.tile_pool(name="poolacc", bufs=1, space="PSUM"))

        def load(parm, shape, dtype):
            t = cpool.tile(list(shape), dtype, tag=parm.name)
            nc.sync.dma_start(out=t[:], in_=parm[:, :])
            return t

        wl_s = [load(wl_p[l], wl_p[l].shape, dt.float16) for l in range(2)]
        wr_s = [load(wr_p[l], wr_p[l].shape, dt.float16) for l in range(2)]
        w2_s = {k: load(p, p.shape, dt.float16) for k, p in w2_p.items()}
        attr_s = [load(attr_p[l], attr_p[l].shape, dt.float16) for l in range(3)]
        blr_s = [load(blr_p[l], blr_p[l].shape, dt.float16) for l in range(3)]
        brr_s = [load(brr_p[l], brr_p[l].shape, dt.float16) for l in range(3)]
        bor_s = [load(bor_p[l], bor_p[l].shape, dt.float16) for l in range(3)]
        ident = load(ident_p, (P, P), dt.float16)
        iota = load(iota_p, (P, P), dt.float16)
        dstcol = load(dstcol_p, (P, NT), dt.float16)
        sgmat = load(sgmat_p, (P, NBLK * G), dt.float16)
        idx0 = load(idx0_p, (P, EP // 16), dt.int16)
        idx1 = load(idx1_p, (P, EP // 16), dt.int16)
        idx2 = load(idx2_p, (P, EP // 16), dt.int16)
        wl1b = load(wl1b_p, (97, 192), dt.float16)
        wr1b = load(wr1b_p, (97, 192), dt.float16)
        idxr0 = load(idxr0_p, (P, EP // 16), dt.int16)
        idxr12 = load(idxr12_p, (P, EP // 16), dt.int16)

        # ---- phase 0: EMB tables (replicated on every core) ----------------
        with tc.tile_pool(name="p0sb", bufs=1) as sb0, \
             tc.tile_pool(name="p0ps", bufs=4, space="PSUM") as ps0:
            embt = sb0.tile([16, 10240], dt.float16, tag="embt")
            nc.sync.dma_start(out=embt[:], in_=embt_p[:, :])
            stage_l = sb0.tile([P, 80, 96], dt.float16, tag="stl")
            stage_r = sb0.tile([P, 80, 96], dt.float16, tag="str")
            for t in range(80):
                lhs = embt[:, t * P:(t + 1) * P]
                for w_s, rep, stg in ((wl_s[0], blr_s[0], stage_l),
                                      (wr_s[0], brr_s[0], stage_r)):
                    pst = ps0.tile([P, 96], dt.float32, tag="embps")
                    nc.tensor.matmul(pst[:], lhs, w_s[:], start=True, stop=True)
                    nc.vector.tensor_tensor(out=stg[:, t, :], in0=pst[:], in1=rep[:],
                                            op=Alu.add)
            nc.sync.dma_start(
                out=embl_d[:, :].rearrange("(t p) c -> p t c", p=P)[:, :, 0:96],
                in_=stage_l[:])
            nc.sync.dma_start(
                out=embr_d[:, :].rearrange("(t p) c -> p t c", p=P)[:, :, 0:96],
                in_=stage_r[:])

        xt_a = cpool.tile([P, NSLOT], dt.float16, tag="xta")
        xt_b = cpool.tile([64, NSLOT], dt.float16, tag="xtb")
        nc.vector.memset(xt_a[96:97, :], 1.0)
        xproj = [cpool.tile([97, 14 * P], dt.float16, tag=f"xproj{i}",
                            name=f"xproj{i}") for i in range(2)]
        for i in range(2):
            nc.vector.memset(xproj[i][96:97, :], 1.0)
        pooled_ps = ppool.tile([64, G], dt.float32, tag="poolps")

        for l, cfg in enumerate(LAYERS):
            F, H, C, elem = cfg["F"], cfg["H"], cfg["C"], cfg["elem"]
            gat_l_src = [embl_d, xlpad_d[1], xlpad_d[2]][l]
            gat_r_src = [embr_d, xrown_d[1], xrown_d[2]][l]
            gat_l_idx = [idx0, idx1, idx2][l]
            gat_r_idx = [idxr0, idxr12, idxr12][l]
            # one gather instruction per pair for the narrow layers, per half
            # for the wide one (SBUF footprint)
            g_tiles = t_blk if elem == 256 else 2 * t_blk

            with tc.tile_pool(name=f"l{l}sb", bufs=SB_BUFS) as sb, \
                 tc.tile_pool(name=f"l{l}gath", bufs=GATH_BUFS) as gpool, \
                 tc.tile_pool(name=f"l{l}st", bufs=(2 if l == 1 else ST_BUFS)) as spool, \
                 tc.tile_pool(name=f"l{l}ps", bufs=OUT_BUFS, space="PSUM") as ps, \
                 tc.tile_pool(name=f"l{l}pss", bufs=1, space="PSUM") as pss:

                def _table_chunk(l, k):
                    cw = CHUNKS[k]
                    cb = CHB[k]
                    co = sum(CHUNKS[i] * NC * P for i in range(k))
                    if l == 0:
                        # ship raw x1^T features; projection happens post-gather
                        nc.sync.dma_start(out=x1ownT_d[k][:, :],
                                          in_=xt_a[0:96, cb * P:(cb + cw) * P])
                        nc.gpsimd.collective_compute(
                            "AllGather", Alu.bypass, replica_groups=replica_groups,
                            ins=[x1ownT_d[k][:, :]], outs=[x1fullT_d[k][:, :]])
                        return
                    nf = LAYERS[l + 1]
                    inF, Fn, elemn = nf["inF"], nf["F"], nf["elem"]
                    stage = sb.tile([P, 14, Fn], dt.float16, tag="stage",
                                    name=f"stage_{l}_{k}")
                    stage_r = sb.tile([P, 14, Fn], dt.float16, tag="stager",
                                      name=f"stager_{l}_{k}")
                    for j in range(cw):
                        nt = cb + j
                        lps = ps.tile([P, 512], dt.float32, tag="outps",
                                      name=f"lps_{l}_{nt}")
                        rps = ps.tile([P, 512], dt.float32, tag="outps",
                                      name=f"rps_{l}_{nt}")
                        if inF <= P:
                            nc.tensor.matmul(lps[:, 0:Fn],
                                             xt_a[0:inF, nt * P:(nt + 1) * P],
                                             wl_s[l + 1][:, :], start=True, stop=True)
                            nc.tensor.matmul(rps[:, 0:Fn],
                                             xt_a[0:inF, nt * P:(nt + 1) * P],
                                             wr_s[l + 1][:, :], start=True, stop=True)
                        else:
                            for wa, wb, tps in ((w2_s["wl2a"], w2_s["wl2b"], lps),
                                                (w2_s["wr2a"], w2_s["wr2b"], rps)):
                                nc.tensor.matmul(tps[:, 0:Fn],
                                                 xt_a[:, nt * P:(nt + 1) * P],
                                                 wa[:, :], start=True, stop=False)
                                nc.tensor.matmul(tps[:, 0:Fn],
                                                 xt_b[:, nt * P:(nt + 1) * P],
                                                 wb[:, :], start=False, stop=True)
                        nc.vector.tensor_tensor(out=stage[:, j, :], in0=lps[:, 0:Fn],
                                                in1=blr_s[l + 1][:, :], op=Alu.add)
                        nc.vector.tensor_tensor(out=stage_r[:, j, :], in0=rps[:, 0:Fn],
                                                in1=brr_s[l + 1][:, :], op=Alu.add)
                    own = xlown_d[l + 1]
                    nc.sync.dma_start(
                        out=own[cb * P:(cb + cw) * P, :]
                            .rearrange("(t p) c -> p t c", p=P),
                        in_=stage[:, 0:cw, :])
                    nc.sync.dma_start(
                        out=xrown_d[l + 1][cb * P:(cb + cw) * P, :]
                            .rearrange("(t p) c -> p t c", p=P)[:, :, 0:Fn],
                        in_=stage_r[:, 0:cw, :])
                    nc.gpsimd.collective_compute(
                        "AllGather", Alu.bypass, replica_groups=replica_groups,
                        ins=[own[cb * P:(cb + cw) * P, :]],
                        outs=[xlfull_d[l + 1][co:co + NC * cw * P, :]])
                    nc.sync.dma_start(
                        out=xlpad_d[l + 1][co:co + NC * cw * P, :]
                            .rearrange("(t p) c -> p t c", p=P)[:, :, 0:Fn],
                        in_=xlfull_d[l + 1][co:co + NC * cw * P, :]
                            .rearrange("(t p) c -> p t c", p=P))

                if l == 1:
                    # xr1own from own x1 (no collective dependency)
                    for g0 in range(0, NBLK, 5):
                        stg = sb.tile([P, 5, 192], dt.float16, tag="pstage",
                                      name=f"xr1st_{g0}")
                        for j in range(5):
                            t = g0 + j
                            pps = ps.tile([P, 512], dt.float32, tag="outps",
                                          name=f"xr1ps_{t}")
                            nc.tensor.matmul(pps[:, 0:192],
                                             xt_a[0:97, t * P:(t + 1) * P],
                                             wr1b[:, :], start=True, stop=True)
                            nc.vector.tensor_copy(out=stg[:, j, :], in_=pps[:, 0:192])
                        nc.sync.dma_start(
                            out=xrown_d[1][g0 * P:(g0 + 5) * P, :]
                                .rearrange("(t p) c -> p t c", p=P)[:, :, 0:192],
                            in_=stg[:])
                    # xl1pad for all cores from the gathered x1^T chunks
                    for k in range(NCH):
                        cw = CHUNKS[k]
                        for c in range(NC):
                            xp = xproj[(k * NC + c) % 2]
                            nc.sync.dma_start(out=xp[0:96, 0:cw * P],
                                              in_=x1fullT_d[k][c * 96:(c + 1) * 96, :])
                            for g0 in range(0, cw, 5):
                                gw = min(5, cw - g0)
                                stg = sb.tile([P, 5, 192], dt.float16, tag="pstage",
                                              name=f"xl1st_{k}_{c}_{g0}")
                                for j in range(gw):
                                    t = g0 + j
                                    pps = ps.tile([P, 512], dt.float32, tag="outps",
                                                  name=f"xl1ps_{k}_{c}_{t}")
                                    nc.tensor.matmul(pps[:, 0:192],
                                                     xp[0:97, t * P:(t + 1) * P],
                                                     wl1b[:, :], start=True, stop=True)
                                    nc.vector.tensor_copy(out=stg[:, j, :], in_=pps[:, 0:192])
                                base = c * NSLOT + (CHB[k] + g0) * P
                                nc.sync.dma_start(
                                    out=xlpad_d[1][base:base + gw * P, :]
                                        .rearrange("(t p) c -> p t c", p=P)[:, :, 0:192],
                                    in_=stg[:, 0:gw, :])

                for pair in range(NPAIR):
                    for gchunk in range(2 * t_blk // g_tiles):
                        gath_l = gpool.tile([P, g_tiles, elem], dt.float16, tag="gathl")
                        gath_r = gpool.tile([P, g_tiles, elem], dt.float16, tag="gathr")
                        ib = pair * (PAIR_E // 16) + gchunk * (g_tiles * P // 16)
                        for ck in range(0, g_tiles, GCK):
                            nt = min(GCK, g_tiles - ck)
                            ni = nt * P
                            nc.gpsimd.dma_gather(
                                gath_l[:, ck:ck + nt, :], gat_l_src[:, :],
                                gat_l_idx[:, ib + ck * 8:ib + ck * 8 + ni // 16],
                                ni, ni, elem, elem_step=elem, single_packet=True)
                            nc.gpsimd.dma_gather(
                                gath_r[:, ck:ck + nt, :], gat_r_src[:, :],
                                gat_r_idx[:, ib + ck * 8:ib + ck * 8 + ni // 16],
                                ni, ni, elem, elem_step=elem, single_packet=True)
                        if g_tiles == 2 * t_blk:
                            gl = [gath_l] * 2
                            gr = [gath_r] * 2
                            goff = [0, t_blk]
                        elif gchunk == 0:
                            gl0, gr0 = gath_l, gath_r
                            continue
                        else:
                            gl = [gl0, gath_l]
                            gr = [gr0, gath_r]
                            goff = [0, 0]

                        for half in range(2):
                            blk = 2 * pair + half
                            sl = goff[half]
                            ghl, ghr = gl[half], gr[half]
                            st_sb = spool.tile([P, t_blk, P], dt.float16, tag="st")
                            if l < 2:
                                nc.sync.dma_start(
                                    out=st_sb[:],
                                    in_=stT_p[:, blk * t_blk * P:(blk + 1) * t_blk * P]
                                        .rearrange("p (t e) -> p t e", e=P))
                            else:
                                nc.vector.tensor_tensor(
                                    out=st_sb[:],
                                    in0=dstcol[:, blk * t_blk:(blk + 1) * t_blk]
                                        .to_broadcast([P, t_blk, P]),
                                    in1=iota[:, :].rearrange("p (o n) -> p o n", o=1)
                                        .to_broadcast([P, t_blk, P]),
                                    op=Alu.is_equal)
                            z = sb.tile([P, t_blk, F], dt.float16, tag="z")
                            nc.vector.tensor_tensor(
                                out=z[:], in0=ghl[:, sl:sl + t_blk, 0:F],
                                in1=ghr[:, sl:sl + t_blk, 0:F], op=Alu.add)
                            eat = sb.tile([P, t_blk, F], dt.float16, tag="eat")
                            nc.scalar.activation(out=eat[:], in_=z[:],
                                                 func=Act.Lrelu, alpha=NEG)
                            nc.vector.tensor_tensor(
                                out=eat[:], in0=eat[:],
                                in1=attr_s[l][:, :].rearrange("p (o f) -> p o f", o=1)
                                    .to_broadcast([P, t_blk, F]),
                                op=Alu.mult)
                            logit = sb.tile([P, t_blk, H], dt.float16, tag="logit")
                            nc.vector.tensor_reduce(
                                out=logit[:],
                                in_=eat[:].rearrange("p g (h c) -> p g h c", h=H),
                                axis=mybir.AxisListType.X, op=Alu.add)
                            wp = sb.tile([P, t_blk, F + H], dt.float16, tag="wp")
                            nc.scalar.activation(out=wp[:, :, F:F + H],
                                                 in_=logit[:], func=Act.Exp)
                            nc.vector.tensor_tensor(
                                out=wp[:, :, 0:F].rearrange("p g (h c) -> p g h c", h=H),
                                in0=ghl[:, sl:sl + t_blk, 0:F]
                                    .rearrange("p g (h c) -> p g h c", h=H),
                                in1=wp[:, :, F:F + H].to_broadcast([P, t_blk, H, C]),
                                op=Alu.mult)
                            out_ps = ps.tile([P, 512], dt.float32, tag="outps")
                            for k in range(t_blk):
                                nc.tensor.matmul(out_ps[:, 0:F + H], st_sb[:, k, :],
                                                 wp[:, k, :],
                                                 start=(k == 0),
                                                 stop=(k == t_blk - 1))

                            # block epilogue: softmax denominators + bias
                            dmax = sb.tile([P, H], dt.float32, tag="dmax")
                            nc.vector.tensor_scalar(out=dmax[:, :],
                                                    in0=out_ps[:, F:F + H],
                                                    scalar1=1e-30, scalar2=None,
                                                    op0=Alu.max)
                            drec = sb.tile([P, H], dt.float32, tag="drec")
                            nc.vector.reciprocal(out=drec[:, :], in_=dmax[:, :])
                            x_blk = sb.tile([P, F], dt.float16, tag="xblk")
                            if H == 1:
                                nc.vector.scalar_tensor_tensor(
                                    out=x_blk[:, :], in0=out_ps[:, 0:F],
                                    scalar=drec[:, 0:1], in1=bor_s[l][:, :],
                                    op0=Alu.mult, op1=Alu.add)
                            else:
                                xtmp = sb.tile([P, F], dt.float32, tag="xtmp")
                                nc.vector.tensor_tensor(
                                    out=xtmp[:, :].rearrange("p (h c) -> p h c", h=H),
                                    in0=out_ps[:, 0:F].rearrange("p (h c) -> p h c", h=H),
                                    in1=drec[:, :].to_broadcast([P, H, C]),
                                    op=Alu.mult)
                                nc.vector.tensor_tensor(out=x_blk[:, :], in0=xtmp[:, :],
                                                        in1=bor_s[l][:, :], op=Alu.add)

                            if l < 2:
                                fa = min(F, P)
                                xt_ps = pss.tile([P, 1024], dt.float16, tag="sps")
                                nc.tensor.transpose(xt_ps[0:fa, 0:P], x_blk[:, 0:fa],
                                                    ident[:])
                                nc.vector.tensor_copy(
                                    out=xt_a[0:fa, blk * P:(blk + 1) * P],
                                    in_=xt_ps[0:fa, 0:P])
                                if F > P:
                                    xt_ps2 = pss.tile([P, 1024], dt.float16, tag="sps")
                                    nc.tensor.transpose(xt_ps2[0:F - P, 0:P],
                                                        x_blk[:, P:F], ident[:])
                                    nc.vector.tensor_copy(
                                        out=xt_b[0:F - P, blk * P:(blk + 1) * P],
                                        in_=xt_ps2[0:F - P, 0:P])
                                if (blk + 1) in CHB[1:]:
                                    _table_chunk(l, CHB.index(blk + 1) - 1)
                            else:
                                nc.tensor.matmul(pooled_ps[:, :], x_blk[:, 0:64],
                                                 sgmat[:, blk * G:(blk + 1) * G],
                                                 start=(blk == 0),
                                                 stop=(blk == NBLK - 1))

        # ---- output: per-core pooled partial sums (head runs on host) -------
        with tc.tile_pool(name="head", bufs=1) as hb:
            pooled_sb = hb.tile([64, G], dt.float32)
            nc.scalar.copy(out=pooled_sb[:, :], in_=pooled_ps[:, :])
            nc.sync.dma_start(out=out_p[:, :], in_=pooled_sb[:, :])

    if not nc.is_finalized():
        nc.finalize()
    return nc


def kernel(**inputs):
    t_blk, cores, shared = _host_prep(inputs)
    nc = _build_program(t_blk)

    from concourse.bass_utils import run_bass_kernel_spmd
    in_maps = []
    for c in range(NC):
        m = dict(shared)
        m.update(cores[c])
        in_maps.append(m)
    res = run_bass_kernel_spmd(nc, in_maps, list(range(NC)))
    pooled = np.sum([np.asarray(res.results[c]["out"]) for c in range(NC)], axis=0)
    return _host_head(pooled, inputs)


def _host_head(pooled, inputs):
    """pooled: [64, G] sum of per-core partials (already mean-scaled)."""
    g = pooled.T.astype(np.float32)                      # [G, 64]
    h = np.concatenate([g, np.asarray(inputs["demographics"], f32)], axis=1)
    h = np.maximum(h @ np.asarray(inputs["Wc1"], f32)
                   + np.asarray(inputs["bc1"], f32), 0.0)
    out = h @ np.asarray(inputs["Wc2"], f32) + np.asarray(inputs["bc2"], f32)
    return np.ascontiguousarray(out).astype(np.float32)


if __name__ == "__main__":
    sys.path.insert(0, os.path.dirname(os.path.abspath(__file__)))
    import reference
    inp = {k: np.asarray(v) for k, v in reference.setup_inputs().items()}
    exp = np.asarray(reference.reference(**inp))
    act = kernel(**inp)
    err = np.abs(act - exp).max() / (np.abs(exp).max() + 1e-12)
    print("rel err:", err)
